# revision 2
# baseline (speedup 1.0000x reference)
"""MoE block (8 experts, top-2, + shared expert) on 8 trn2 NeuronCores.

Strategy (FF-sharded tensor parallelism, host dispatch/combine):
  - Host computes gate logits/softmax/top-2 (0.03% of total FLOPs).
  - Every core receives the SAME activation matrix xt = [all 4096 tokens
    (shared-expert pass) | expert-0's routed tokens | ... | expert-7's],
    and a distinct 512-wide slice of the FF dim of EVERY weight matrix
    (8 experts + shared = 9 groups, 18MB bf16, all SBUF-resident).
    gelu is elementwise over FF, so y = sum_c gelu(x@W1[:,c])@W2[c,:]
    decomposes exactly; the host sums the 8 partial outputs in fp32.
  - Per-core work is identical regardless of routing (no padding to the
    max expert count, no load imbalance): 12288+pad cols x 1/8 of FF.
  - Matmuls are bf16 with fp32 PSUM accumulation; feature-major layout
    ([D, tokens]) avoids all on-device transposes. Outputs return bf16
    (partials are summed in fp32 on host; quantization adds ~1e-3 rel).

Layout per core (SPMD, same program all 8 cores):
  inputs : xt [1024, NT] bf16 (cols = [4096 shared | routed by expert])
           w1_g{i} [1024, 512] bf16, w2_g{i} [512, 1024] bf16 for the
           9 weight groups (group 0 = shared, 1..8 = experts, permuted
           so the smallest remainder chunk lands last)
           ball [128, 36] fp32 (gelu biases: col 4i+f = group i, f-tile f)
  output : yt [1024, NT] bf16 (partial sums over this core's FF slice)

DMA: batched 3D access-pattern transfers (1-2 dma_starts per tensor);
each dma_start costs ~0.65us of Sync-sequencer issue time, so fewer+
bigger transfers start compute earlier and cut issue overhead.
"""

import numpy as np
import ml_dtypes

import concourse.bass as bass
import concourse.bacc as bacc
from concourse import mybir
from concourse.tile import TileContext
from concourse.bass_utils import run_bass_kernel_spmd

D = 1024
FF = 4096
E = 8
TOPK = 2
B, L = 4, 1024
T = B * L
NCORES = 8
P = 128
DT = D // P        # 8 k-tiles over D
FSL = FF // NCORES # 512 FF columns per core
FT = FSL // P      # 4 f-tiles per core slice
NG = E + 1         # weight groups: 0 = shared, 1..8 = experts

_BF16 = mybir.dt.bfloat16
_F32 = mybir.dt.float32

_program_cache: dict[tuple, object] = {}

# test harness hooks: extra kwargs for run_bass_kernel_spmd (e.g. trace=True)
# and the last BassKernelResults for profiling. Unused in normal grading runs.
TRACE_KWARGS: dict = {}
last_results = None

N_WARM = 40  # dummy matmuls bridging the ~7us DMA delivery after kernel start


def _chunks_for_width(w: int) -> list[int]:
    n_full, rem = divmod(w, 512)
    return [512] * n_full + ([rem] if rem else [])


def _build_program(widths: tuple[int, ...]):
    """One SPMD program: 9 weight groups over [4096 | widths] columns."""
    NT = 4096 + sum(widths)
    nc = bacc.Bacc()

    xt = nc.dram_tensor("xt", [D, NT], _BF16, kind="ExternalInput")
    w1g = [nc.dram_tensor(f"w1_g{i}", [D, FSL], _BF16, kind="ExternalInput")
           for i in range(NG)]
    w2g = [nc.dram_tensor(f"w2_g{i}", [FSL, D], _BF16, kind="ExternalInput")
           for i in range(NG)]
    ballr = nc.dram_tensor("ball", [P, NG * FT], _F32, kind="ExternalInput")
    yt = nc.dram_tensor("yt", [D, NT], _BF16, kind="ExternalOutput")

    # chunk list: (group, col_offset, width)
    chunks = [(0, 512 * k, 512) for k in range(8)]
    off = 4096
    for j, w in enumerate(widths):
        for cw in _chunks_for_width(w):
            chunks.append((1 + j, off, cw))
            off += cw
    assert off == NT
    NCH = len(chunks)

    with TileContext(nc) as tc:
        with (
            tc.tile_pool(name="wpool", bufs=1) as wpool,
            tc.tile_pool(name="xpool", bufs=2) as xpool,
            tc.tile_pool(name="hpool", bufs=8) as hpool,
            tc.tile_pool(name="ypool", bufs=2) as ypool,
            tc.tile_pool(name="bpool", bufs=1) as bpool,
            tc.tile_pool(name="psum", bufs=4, space="PSUM") as psum,
        ):
            def load_w1(g):
                # two column-halves so the first f-tiles' weights arrive
                # (and unblock matmul) before the full megabyte lands
                ta = wpool.tile([P, DT, FSL // 2], _BF16, tag=f"w1a_{g}",
                                name=f"w1a_{g}")
                tb = wpool.tile([P, DT, FSL // 2], _BF16, tag=f"w1b_{g}",
                                name=f"w1b_{g}")
                src = w1g[g][:, :].rearrange("(d p) n -> p d n", p=P)
                nc.sync.dma_start(ta, src[:, :, :FSL // 2])
                nc.sync.dma_start(tb, src[:, :, FSL // 2:])
                return (ta, tb)

            def w1_lhsT(t, d, f):
                ta, tb = t
                half, fi = divmod(f, FT // 2)
                src = ta if half == 0 else tb
                return src[:, d, fi * P:(fi + 1) * P]

            def load_w2(g):
                t = wpool.tile([P, FT, D], _BF16, tag=f"w2_{g}", name=f"w2_{g}")
                nc.sync.dma_start(
                    t, w2g[g][:, :].rearrange("(f p) n -> p f n", p=P))
                return t

            def load_x(ci):
                _, coff, N = chunks[ci]
                t = xpool.tile([P, DT, 512], _BF16, tag="x", name=f"x_{ci}")
                t = t[:, :, :N]
                nc.sync.dma_start(
                    t, xt[:, coff:coff + N].rearrange("(d p) n -> p d n", p=P))
                return t

            # PE warm-up: dummy matmuls on a zeroed tile keep the PE busy
            # across the DMA delivery latency so the HAM clock-gate is at
            # 8/8 (2.4 GHz) when real matmuls issue.
            warm = bpool.tile([P, P + 512], _BF16, tag="warm", name="warm")
            nc.any.memset(warm[:, :], 0.0)
            wps = psum.tile([P, 512], _F32, tag="py", name="pwarm")
            for _ in range(N_WARM):
                nc.tensor.matmul(wps, lhsT=warm[:, :P], rhs=warm[:, P:],
                                 start=True, stop=True)

            # critical prefetch: first w1 half + first x chunk
            ball = bpool.tile([P, NG * FT], _F32, tag="ball", name="ball")
            nc.sync.dma_start(ball, ballr[:, :])
            w1t: dict[int, tuple] = {}
            w2t: dict[int, object] = {}
            w1t[0] = load_w1(0)
            x_next = load_x(0)
            w2t[0] = load_w2(0)

            for ci, (g, coff, N) in enumerate(chunks):
                xts = x_next
                if ci + 1 < NCH:
                    x_next = load_x(ci + 1)
                # stream expert weights during the shared phase (2 groups
                # per shared chunk keeps per-queue backlog small so x
                # prefetches aren't stuck behind weight megabytes)
                if ci < 4:
                    for g2 in (2 * ci + 1, 2 * ci + 2):
                        w1t[g2] = load_w1(g2)
                        w2t[g2] = load_w2(g2)

                hts = []
                for f in range(FT):
                    ph = psum.tile([P, 512], _F32, tag="ph", name="ph")[:, :N]
                    for d in range(DT):
                        nc.tensor.matmul(
                            ph,
                            lhsT=w1_lhsT(w1t[g], d, f),
                            rhs=xts[:, d, :],
                            start=(d == 0),
                            stop=(d == DT - 1),
                        )
                    ht = hpool.tile([P, 512], _BF16, tag="h", name="h")[:, :N]
                    nc.scalar.activation(
                        ht, ph, mybir.ActivationFunctionType.Gelu,
                        bias=ball[:, g * FT + f:g * FT + f + 1],
                    )
                    hts.append(ht)

                yo = ypool.tile([P, DT, 512], _BF16, tag="y", name="y")
                yo = yo[:, :, :N]
                for d in range(DT):
                    py = psum.tile([P, 512], _F32, tag="py", name="py")[:, :N]
                    for f in range(FT):
                        nc.tensor.matmul(
                            py,
                            lhsT=w2t[g][:, f, d * P:(d + 1) * P],
                            rhs=hts[f],
                            start=(f == 0),
                            stop=(f == FT - 1),
                        )
                    nc.vector.tensor_copy(yo[:, d, :], py)
                nc.sync.dma_start(
                    yt[:, coff:coff + N].rearrange("(d p) n -> p d n", p=P),
                    yo)

    nc.finalize()
    return nc


def _get_program(widths: tuple[int, ...]):
    if widths not in _program_cache:
        _program_cache[widths] = _build_program(widths)
    return _program_cache[widths]


def _route(xf: np.ndarray, W_gate: np.ndarray):
    """Replicate the reference gate in float64 (selection margins are ~1e-5,
    far above fp32 rounding, so the top-2 sets match the fp32 reference)."""
    logits = xf.astype(np.float64) @ W_gate.astype(np.float64)
    m = logits.max(axis=-1, keepdims=True)
    p = np.exp(logits - m)
    p /= p.sum(axis=-1, keepdims=True)
    top_i = np.argsort(-p, axis=-1, kind="stable")[:, :TOPK]
    top_v = np.take_along_axis(p, top_i, axis=-1)
    top_v = top_v / top_v.sum(axis=-1, keepdims=True)
    return top_i, top_v.astype(np.float32)


def kernel(x, W_gate, W1, b1, W2, b2, Ws1, bs1, Ws2, bs2):
    x = np.asarray(x, np.float32)
    xf = x.reshape(T, D)
    top_i, top_v = _route(xf, np.asarray(W_gate, np.float32))

    # per-expert token lists
    idx = [np.nonzero((top_i == e).any(axis=1))[0] for e in range(E)]
    wgt = []
    for e in range(E):
        sel = top_i[idx[e]] == e  # [cnt, K] exactly one True per row
        wgt.append(top_v[idx[e]][sel].astype(np.float32))
    counts = np.array([len(i) for i in idx])
    we = [int(-(-c // 8) * 8) for c in counts]  # expert col widths, 8-aligned

    # group order: experts with the largest tail chunk first, so the
    # globally-last chunk (the exec-time tail) is the smallest one
    def tailsz(e):
        r = we[e] % 512
        return r if r else 512
    perm = sorted(range(E), key=lambda e: -tailsz(e))
    widths = tuple(we[e] for e in perm)
    NT = 4096 + sum(widths)

    xbf = xf.astype(ml_dtypes.bfloat16)
    cols = np.zeros(NT, np.int64)
    cols[:T] = np.arange(T)
    goff = []
    off = 4096
    for j, e in enumerate(perm):
        goff.append(off)
        cols[off:off + counts[e]] = idx[e]
        off += we[e]
    xtc = np.ascontiguousarray(xbf[cols].T)

    W1 = np.asarray(W1, np.float32).astype(ml_dtypes.bfloat16)
    W2 = np.asarray(W2, np.float32).astype(ml_dtypes.bfloat16)
    Ws1b = np.asarray(Ws1, np.float32).astype(ml_dtypes.bfloat16)
    Ws2b = np.asarray(Ws2, np.float32).astype(ml_dtypes.bfloat16)
    b1f = np.asarray(b1, np.float32)
    bs1f = np.asarray(bs1, np.float32)

    in_maps = []
    for c in range(E):
        sl = slice(c * FSL, (c + 1) * FSL)
        m = {"xt": xtc}
        ball = np.zeros((P, NG * FT), np.float32)
        ball[:, :FT] = bs1f[sl].reshape(FT, P).T
        m["w1_g0"] = np.ascontiguousarray(Ws1b[:, sl])
        m["w2_g0"] = np.ascontiguousarray(Ws2b[sl, :])
        for j, e in enumerate(perm):
            m[f"w1_g{1 + j}"] = np.ascontiguousarray(W1[e][:, sl])
            m[f"w2_g{1 + j}"] = np.ascontiguousarray(W2[e][sl, :])
            ball[:, (1 + j) * FT:(2 + j) * FT] = b1f[e][sl].reshape(FT, P).T
        m["ball"] = ball
        in_maps.append(m)

    nc = _get_program(widths)
    global last_results
    last_results = run_bass_kernel_spmd(
        nc, in_maps, list(range(NCORES)), **TRACE_KWARGS)
    res = last_results.results

    ysum = np.zeros((D, NT), np.float32)
    for c in range(E):
        ysum += np.asarray(res[c]["yt"], dtype=np.float32)

    out = np.ascontiguousarray(ysum[:, :T].T)  # shared expert, all tokens
    for j, e in enumerate(perm):
        cnt = counts[e]
        out[idx[e]] += wgt[e][:, None] * ysum[:, goff[j]:goff[j] + cnt].T

    # b2/bs2 enter linearly; add on host (zeros in this problem's inputs)
    b2 = np.asarray(b2, np.float32)
    bs2 = np.asarray(bs2, np.float32)
    combine = np.zeros((T, E), np.float32)
    np.put_along_axis(combine, top_i, top_v, axis=1)
    out += combine @ b2 + bs2

    return out.reshape(B, L, D)
